# revision 10
# baseline (speedup 1.0000x reference)
"""GCNConv on 8 trn2 NeuronCores.

out = segment_sum(edge_vals * (x @ weight)[edge_cols], edge_rows) + bias

Strategy (v2):
  - Nodes sharded 8 ways: 12800 padded rows/core (12500 real).
  - The h = x@W table is built in 4 CHUNKS of 3200 rows/core; after each
    chunk is projected it is AllGather'd (chunk-major table layout), so
    the gather phase can start on sub-table c right after AllGather_c
    instead of waiting for the whole collective.
  - Sub-table c = 8 cores x 3200 rows = 25600 rows (< 32768, int16-safe
    dma_gather indices).
  - Phase 3 on each core: edges partitioned by destination owner, sorted
    by (chunk, dest block). dma_gather fetches h[col] rows in 4096-index
    calls (dynamic_dma_scratch_size=64KB -> 4096-descriptor rings, one
    call per ring, round-robin over 4 SWDGE queues).
  - The weighted segment-sum runs on the PE: per 128-edge tile a scatter
    matrix S[e, r] = val[e] * (rel[e] == r) is built ON-CHIP by the
    vector engine (iota-compare + multiply, bulk 32 tiles per op pair)
    instead of streaming 51MB of host-precomputed matrices from DRAM.
  - Output accumulates in an SBUF fp16 buffer across the 4 chunks
    (bias folded into the chunk-0 add) and is written out as fp16.

Per-(block, chunk) tile counts are maxed across cores so all 8 cores run
one identical SPMD program (only input data differs).
"""

import numpy as np

import concourse.bacc as bacc
import concourse.bass as bass
import concourse.mybir as mybir
import concourse.tile as tile
from concourse.bass_utils import run_bass_kernel_spmd

N_NODES = 100000
N_EDGES = 1600000
IN_F = 256
OUT_F = 128
N_CORES = 8
P = 128

RREAL = N_NODES // N_CORES              # 12500 real rows per core
N_CHUNK = 4                             # projection / AllGather chunks
CH_R = 3200                             # chunk rows per core (25 blocks)
ROWS_PER_CORE = N_CHUNK * CH_R          # 12800 padded rows per core
CH_BLOCKS = CH_R // P                   # 25 projection blocks per chunk
SUB_ROWS = CH_R * N_CORES               # 25600 gather sub-table rows (<32768)
N_PAD = ROWS_PER_CORE * N_CORES         # 102400
AGG_BLOCKS = 100                        # dest blocks per core (125 rows each)
BLOCK_R = RREAL // AGG_BLOCKS           # 125
import os

CALL_T = int(os.environ.get("K_CALL_T", "32"))  # 128-edge tiles per gather call
N_QUEUES = 4
SCRATCH = int(os.environ.get("K_SCRATCH", str(1 << 16)))  # swdge scratch bytes

_F32 = mybir.dt.float32
_F16 = mybir.dt.float16
_I16 = mybir.dt.int16

_cache = {}
TRACE = False
LAST_EXEC_NS = None


def _prep_host(x, edge_rows, edge_cols, edge_vals, weight, bias):
    """Shard + sort edges, build per-core padded meta arrays."""
    xT = np.zeros((IN_F, N_PAD), dtype=np.float16)
    xt_f16 = x.astype(np.float16).T
    for c in range(N_CORES):
        xT[:, c * ROWS_PER_CORE : c * ROWS_PER_CORE + RREAL] = xt_f16[
            :, c * RREAL : (c + 1) * RREAL
        ]
    xT = xT.reshape(2, P, N_PAD)
    w_f = np.ascontiguousarray(weight.astype(np.float16)).reshape(2, P, OUT_F)
    bias_b = np.broadcast_to(bias.astype(np.float16), (P, OUT_F)).copy()
    iota = np.broadcast_to(
        np.arange(P, dtype=np.float16), (P, P)
    ).copy()  # iota[p, r] = r

    owner = edge_rows // RREAL
    local_r = (edge_rows - owner * RREAL).astype(np.int64)
    block = local_r // BLOCK_R
    rel = (local_r - block * BLOCK_R).astype(np.int64)  # < 125

    c_owner = edge_cols // RREAL
    lc = (edge_cols - c_owner * RREAL).astype(np.int64)
    chunk = lc // CH_R                                   # 0..3
    sub_idx = (c_owner * CH_R + (lc - chunk * CH_R)).astype(np.int16)

    # cell = chunk-major so gather streams match the AllGather pipeline
    cell = chunk * AGG_BLOCKS + block
    N_CELLS = N_CHUNK * AGG_BLOCKS

    per_core = []
    counts = np.zeros((N_CORES, N_CELLS), dtype=np.int64)
    for c in range(N_CORES):
        m = owner == c
        cell_m = cell[m]
        order = np.argsort(cell_m, kind="stable")
        per_core.append(
            (cell_m[order], sub_idx[m][order], rel[m][order], edge_vals[m][order])
        )
        counts[c] = np.bincount(cell_m, minlength=N_CELLS)

    # uniform-across-cores tile counts per cell
    T = ((counts.max(axis=0) + P - 1) // P).astype(np.int64)  # [N_CELLS]
    for b in range(AGG_BLOCKS):
        if T[b] == 0:
            T[b] = 1  # chunk-0 cell always exists (carries the bias add)
    slot_of = np.zeros(N_CELLS + 1, dtype=np.int64)
    np.cumsum(T, out=slot_of[1:])
    n_tiles = int(slot_of[-1])
    chunk_slot0 = [int(slot_of[c * AGG_BLOCKS]) for c in range(N_CHUNK)] + [n_tiles]

    # per-chunk gather calls of CALL_T tiles
    calls = []  # (chunk, tile_offset_in_chunk, ct)
    for c in range(N_CHUNK):
        nt_c = chunk_slot0[c + 1] - chunk_slot0[c]
        o = 0
        while o < nt_c:
            ct = min(CALL_T, nt_c - o)
            calls.append((c, o, ct))
            o += ct

    idx_hosts, rel_hosts, val_hosts = [], [], []
    for c in range(N_CORES):
        cell_c, sub_c, rel_c, val_c = per_core[c]
        starts = np.zeros(N_CELLS + 1, dtype=np.int64)
        np.cumsum(counts[c], out=starts[1:])
        n_e = len(cell_c)
        pos_in_cell = np.arange(n_e) - starts[cell_c]
        slot_e = slot_of[cell_c] + (pos_in_cell >> 7)
        part_e = (pos_in_cell & 127).astype(np.int64)

        idx_slots = np.zeros((n_tiles, P), dtype=np.int16)
        idx_slots[slot_e, part_e] = sub_c
        rel_slots = np.zeros((n_tiles, P), dtype=np.float16)
        rel_slots[slot_e, part_e] = rel_c.astype(np.float16)
        val_slots = np.zeros((n_tiles, P), dtype=np.float16)
        val_slots[slot_e, part_e] = val_c.astype(np.float16)

        # per-chunk idx streams wrapped into 16 partitions
        idx_ch = []
        for ch in range(N_CHUNK):
            s0, s1 = chunk_slot0[ch], chunk_slot0[ch + 1]
            stream = idx_slots[s0:s1].reshape(-1)
            idx_ch.append(np.ascontiguousarray(stream.reshape(-1, 16).T))  # [16, nt*8]
        idx_hosts.append(idx_ch)
        rel_hosts.append(np.ascontiguousarray(rel_slots.T))  # [128, n_tiles]
        val_hosts.append(np.ascontiguousarray(val_slots.T))

    meta = dict(
        T=T,
        slot_of=slot_of,
        chunk_slot0=chunk_slot0,
        n_tiles=n_tiles,
        calls=calls,
    )
    in_maps = []
    for c in range(N_CORES):
        im = {
            "xT": np.ascontiguousarray(
                xT[:, :, c * ROWS_PER_CORE : (c + 1) * ROWS_PER_CORE]
            ),
            "w": w_f,
            "biasb": bias_b,
            "iota": iota,
            "rel16": rel_hosts[c],
            "val16": val_hosts[c],
        }
        for ch in range(N_CHUNK):
            im[f"idx{ch}"] = idx_hosts[c][ch]
        in_maps.append(im)
    return meta, in_maps


def _build_program(meta):
    T = meta["T"]
    slot_of = meta["slot_of"]
    chunk_slot0 = meta["chunk_slot0"]
    n_tiles = meta["n_tiles"]
    calls = meta["calls"]

    nc = bacc.Bacc(
        "TRN2",
        debug=False,
        num_devices=N_CORES,
        num_swdge_queues=N_QUEUES,
        dynamic_dma_scratch_size=SCRATCH,
    )

    xT_ap = nc.dram_tensor("xT", [2, P, ROWS_PER_CORE], _F16, kind="ExternalInput").ap()
    w_ap = nc.dram_tensor("w", [2, P, OUT_F], _F16, kind="ExternalInput").ap()
    biasb_ap = nc.dram_tensor("biasb", [P, OUT_F], _F16, kind="ExternalInput").ap()
    iota_ap = nc.dram_tensor("iota", [P, P], _F16, kind="ExternalInput").ap()
    rel_ap = nc.dram_tensor("rel16", [P, n_tiles], _F16, kind="ExternalInput").ap()
    val_ap = nc.dram_tensor("val16", [P, n_tiles], _F16, kind="ExternalInput").ap()
    idx_aps = []
    for ch in range(N_CHUNK):
        nt_c = chunk_slot0[ch + 1] - chunk_slot0[ch]
        idx_aps.append(
            nc.dram_tensor(f"idx{ch}", [16, nt_c * 8], _I16, kind="ExternalInput").ap()
        )
    out_ap = nc.dram_tensor("out", [RREAL, OUT_F], _F16, kind="ExternalOutput").ap()

    h_shard = nc.dram_tensor("h_shard", [ROWS_PER_CORE, OUT_F], _F16).ap()
    h_full = nc.dram_tensor("h_full", [N_PAD, OUT_F], _F16, addr_space="Shared").ap()

    with tile.TileContext(nc) as tc:
        with (
            tc.tile_pool(name="meta", bufs=1) as pm,
            tc.tile_pool(name="idxp", bufs=2) as pidx,
            tc.tile_pool(name="xt", bufs=2) as pxt,
            tc.tile_pool(name="p1h", bufs=4) as p1h,
            tc.tile_pool(name="gat", bufs=5) as pg,
            tc.tile_pool(name="spool", bufs=2) as psp,
            tc.tile_pool(name="psum3", bufs=8, space="PSUM") as pp,
        ):
            # ---------------- static meta loads ----------------
            w_sb = [
                pm.tile([P, OUT_F], _F16, tag=f"w{k}", name=f"w_sb{k}")
                for k in range(2)
            ]
            for k in range(2):
                nc.sync.dma_start(out=w_sb[k][:], in_=w_ap[k])
            biasb_sb = pm.tile([P, OUT_F], _F16, name="biasb_sb")
            nc.sync.dma_start(out=biasb_sb[:], in_=biasb_ap[:])
            iota_sb = pm.tile([P, P], _F16, name="iota_sb")
            nc.sync.dma_start(out=iota_sb[:], in_=iota_ap[:])
            rel_sb = pm.tile([P, n_tiles], _F16, name="rel_sb")
            nc.sync.dma_start(out=rel_sb[:], in_=rel_ap[:])
            val_sb = pm.tile([P, n_tiles], _F16, name="val_sb")
            nc.sync.dma_start(out=val_sb[:], in_=val_ap[:])
            # fp16 output accumulator (bias folded in at chunk 0)
            out_sb = pm.tile([P, AGG_BLOCKS * OUT_F], _F16, name="out_sb")

            # per-chunk gather index tiles ([128, nt*8]; the 8 gpsimd ucode
            # cores each read their own 16-partition band -> replicate x8)
            idx_sb = []
            for ch in range(N_CHUNK):
                nt_c = chunk_slot0[ch + 1] - chunk_slot0[ch]
                it = pidx.tile([P, nt_c * 8], _I16, tag="idx", name=f"idx_sb{ch}")
                for r in range(8):
                    nc.sync.dma_start(
                        out=it[16 * r : 16 * (r + 1), :], in_=idx_aps[ch][:]
                    )
                idx_sb.append(it)

            # ---------------- phase 1: h = x @ W, chunked + AllGather ------
            for ch in range(N_CHUNK):
                xt_sb = [
                    pxt.tile([P, CH_R], _F16, tag=f"xt{k}", name=f"xt{ch}_{k}")
                    for k in range(2)
                ]
                for k in range(2):
                    nc.sync.dma_start(
                        out=xt_sb[k][:], in_=xT_ap[k][:, ch * CH_R : (ch + 1) * CH_R]
                    )
                for nt in range(CH_BLOCKS):
                    ph = pp.tile([P, OUT_F], _F32, tag="ps")
                    for k in range(2):
                        nc.tensor.matmul(
                            ph[:],
                            lhsT=xt_sb[k][:, nt * P : (nt + 1) * P],
                            rhs=w_sb[k][:],
                            start=(k == 0),
                            stop=(k == 1),
                        )
                    hb = p1h.tile([P, OUT_F], _F16)
                    nc.scalar.copy(out=hb[:], in_=ph[:])
                    r0 = ch * CH_R + nt * P
                    nc.sync.dma_start(out=h_shard[r0 : r0 + P, :], in_=hb[:])

            # AllGather chunk-by-chunk (gpsimd stream: AG0..AG3 then gathers)
            for ch in range(N_CHUNK):
                nc.gpsimd.collective_compute(
                    "AllGather",
                    mybir.AluOpType.bypass,
                    replica_groups=[list(range(N_CORES))],
                    ins=[h_shard[ch * CH_R : (ch + 1) * CH_R, :]],
                    outs=[h_full[ch * SUB_ROWS : (ch + 1) * SUB_ROWS, :]],
                )

            # ---------------- phase 3: gather + scatter-matmul -------------
            # gather calls (gpsimd stream, round-robin queues)
            g_tiles = {}  # (chunk, call_k) -> (g_tile, ct)
            call_k_of = {}
            kk = {c: 0 for c in range(N_CHUNK)}
            for call_no, (ch, o, ct) in enumerate(calls):
                k = kk[ch]
                kk[ch] = k + 1
                g = pg.tile([P, ct, P], _F16, tag="gather", name=f"g_{ch}_{k}")
                nc.gpsimd.dma_gather(
                    out_ap=g[:],
                    in_ap=h_full[ch * SUB_ROWS : (ch + 1) * SUB_ROWS, :],
                    idxs_ap=idx_sb[ch][:, o * 8 : (o + ct) * 8],
                    num_idxs=ct * P,
                    num_idxs_reg=ct * P,
                    elem_size=OUT_F,
                    queue_num=call_no % N_QUEUES,
                )
                g_tiles[(ch, k)] = (g, ct)

            # consumption: per chunk, S-builds interleaved with block chains
            for ch in range(N_CHUNK):
                s0_c = chunk_slot0[ch]
                nt_c = chunk_slot0[ch + 1] - s0_c
                n_calls = (nt_c + CALL_T - 1) // CALL_T

                # block b's chain ends in call end_call[b]
                ends = []
                for b in range(AGG_BLOCKS):
                    cell = ch * AGG_BLOCKS + b
                    if T[cell] == 0:
                        ends.append(-1)
                        continue
                    last_slot = int(slot_of[cell]) + int(T[cell]) - 1
                    ends.append((last_slot - s0_c) // CALL_T)

                s_tiles = {}
                next_b = 0

                def emit_block(b):
                    cell = ch * AGG_BLOCKS + b
                    tcnt = int(T[cell])
                    ph3 = pp.tile([P, OUT_F], _F32, tag="ps", name=f"ph3_{ch}_{b}")
                    for i in range(tcnt):
                        slot = int(slot_of[cell]) + i
                        rel_k = (slot - s0_c) // CALL_T
                        pos = (slot - s0_c) % CALL_T
                        g, _ct = g_tiles[(ch, rel_k)]
                        s_t = s_tiles[rel_k]
                        nc.tensor.matmul(
                            ph3[:],
                            lhsT=s_t[:, pos, :],
                            rhs=g[:, pos, :],
                            start=(i == 0),
                            stop=(i == tcnt - 1),
                        )
                    ob = out_sb[:, b * OUT_F : (b + 1) * OUT_F]
                    if ch == 0:
                        nc.vector.tensor_add(out=ob, in0=ph3[:], in1=biasb_sb[:])
                    else:
                        nc.vector.tensor_add(out=ob, in0=ph3[:], in1=ob)
                    if ch == N_CHUNK - 1:
                        nc.sync.dma_start(
                            out=out_ap[b * BLOCK_R : (b + 1) * BLOCK_R, :],
                            in_=out_sb[0:BLOCK_R, b * OUT_F : (b + 1) * OUT_F],
                        )

                for k in range(n_calls):
                    # build S for call k's tiles (bulk iota-compare * val)
                    ct = g_tiles[(ch, k)][1]
                    s_t = psp.tile([P, ct, P], _F16, tag="smat", name=f"s_{ch}_{k}")
                    sl0 = s0_c + k * CALL_T
                    nc.vector.tensor_tensor(
                        out=s_t[:],
                        in0=iota_sb[:].unsqueeze(1).broadcast_to([P, ct, P]),
                        in1=rel_sb[:, sl0 : sl0 + ct].unsqueeze(2).broadcast_to(
                            [P, ct, P]
                        ),
                        op=mybir.AluOpType.is_equal,
                    )
                    nc.vector.tensor_tensor(
                        out=s_t[:],
                        in0=s_t[:],
                        in1=val_sb[:, sl0 : sl0 + ct].unsqueeze(2).broadcast_to(
                            [P, ct, P]
                        ),
                        op=mybir.AluOpType.mult,
                    )
                    s_tiles[k] = s_t
                    # emit chains for blocks that ended in call k-1
                    while next_b < AGG_BLOCKS and ends[next_b] < k:
                        if ends[next_b] >= 0:
                            emit_block(next_b)
                        next_b += 1
                while next_b < AGG_BLOCKS:
                    if ends[next_b] >= 0:
                        emit_block(next_b)
                    next_b += 1
                if ch == N_CHUNK - 1:
                    # blocks with no chunk-3 edges still need their out write
                    for b in range(AGG_BLOCKS):
                        if ends[b] < 0:
                            nc.sync.dma_start(
                                out=out_ap[b * BLOCK_R : (b + 1) * BLOCK_R, :],
                                in_=out_sb[0:BLOCK_R, b * OUT_F : (b + 1) * OUT_F],
                            )

    nc.compile()
    return nc


def kernel(x, edge_rows, edge_cols, edge_vals, weight, bias):
    x = np.asarray(x)
    edge_rows = np.asarray(edge_rows)
    edge_cols = np.asarray(edge_cols)
    edge_vals = np.asarray(edge_vals)
    weight = np.asarray(weight)
    bias = np.asarray(bias)

    meta, in_maps = _prep_host(x, edge_rows, edge_cols, edge_vals, weight, bias)

    key = meta["T"].tobytes()
    if key not in _cache:
        _cache[key] = _build_program(meta)
    nc = _cache[key]

    res = run_bass_kernel_spmd(nc, in_maps, list(range(N_CORES)), trace=TRACE)
    global LAST_EXEC_NS
    LAST_EXEC_NS = res.exec_time_ns
    out = np.concatenate([res.results[c]["out"] for c in range(N_CORES)], axis=0)
    return out.astype(np.float32)


# revision 11
# speedup vs baseline: 1.7178x; 1.7178x over previous
"""GCNConv on 8 trn2 NeuronCores.

out = segment_sum(edge_vals * (x @ weight)[edge_cols], edge_rows) + bias

Strategy (v3 = baseline structure + swdge/f16/pipelining tuning):
  - Nodes (rows of x / out) sharded 8 ways: 12544 rows/core (padded 100352).
  - Phase 1: each core projects its x-shard: h_shard = x_shard @ W (fp16 PE),
    xT streamed in 4 column chunks so matmuls start early.
  - Phase 2: AllGather h shards -> full h table [100352, 128] fp16 in each
    core's DRAM.
  - Phase 3: each core aggregates its own destination rows. Edges are
    partitioned by destination owner on the host, sorted by (dest block,
    source quadrant), padded to 128-edge tiles. dma_gather fetches h[col]
    rows (1024 idx/call - hard ucode ring-entry limit - spread over 4 SWDGE
    queues; dynamic_dma_scratch_size=64KB gives 4096-descriptor rings so
    up to 4 calls per queue are in flight and descriptor generation is not
    gated on DMA drain). The weighted segment-sum runs on the PE: per
    128-edge tile a host-precomputed scatter matrix S[e, r] =
    val[e] * (row[e] == r) (fp16, streamed in bulk from DRAM) is matmul'd
    against the gathered messages, accumulating each 128-row output block
    in PSUM; bias is added on the PSUM->SBUF copy; output is written fp16.
  - Source quadrants exist because dma_gather indices are int16 (<32768):
    the h table is viewed as 4 sub-tables of 25088 rows.

The per-(block, quadrant) tile counts are maxed across cores so all 8 cores
run one identical SPMD program (only input data differs).
"""

import numpy as np

import concourse.bacc as bacc
import concourse.bass as bass
import concourse.mybir as mybir
import concourse.tile as tile
from concourse.bass_utils import run_bass_kernel_spmd

N_NODES = 100000
N_EDGES = 1600000
IN_F = 256
OUT_F = 128
N_CORES = 8
P = 128

BLOCKS_PER_CORE = 98                         # phase-1 projection tiling
ROWS_PER_CORE = BLOCKS_PER_CORE * P          # 12544 (padded shard rows)
RREAL = N_NODES // N_CORES                   # 12500 real rows per core
AGG_BLOCKS = 112                             # aggregation blocks (span <=112 rows)
N_PAD = ROWS_PER_CORE * N_CORES              # 100352
N_QUAD = 4
QUAD_ROWS = N_PAD // N_QUAD                  # 25088 (< 32768, int16-safe)
SG_BLOCKS = 4                                # blocks per gather supergroup
N_SG = (AGG_BLOCKS + SG_BLOCKS - 1) // SG_BLOCKS
CALL_TILES = 8                               # 1024 idx/call (hard ucode limit)
N_QUEUES = 4
SCRATCH = 1 << 16                            # 64KB -> 4096-desc swdge rings
XCHUNKS = 4                                  # phase-1 xT load chunks

_F32 = mybir.dt.float32
_F16 = mybir.dt.float16
_I16 = mybir.dt.int16

_cache = {}
TRACE = False
LAST_EXEC_NS = None


def _prep_host(x, edge_rows, edge_cols, edge_vals, weight, bias):
    """Shard + sort edges, build per-core padded meta arrays."""
    xT = np.zeros((IN_F, N_PAD), dtype=np.float16)
    xt_f16 = x.astype(np.float16).T
    for c in range(N_CORES):
        xT[:, c * ROWS_PER_CORE : c * ROWS_PER_CORE + RREAL] = xt_f16[
            :, c * RREAL : (c + 1) * RREAL
        ]
    xT = xT.reshape(2, P, N_PAD)
    w_f = np.ascontiguousarray(weight.astype(np.float16)).reshape(2, P, OUT_F)
    bias_b = np.broadcast_to(bias.astype(np.float16), (P, OUT_F)).copy()

    st = (np.arange(AGG_BLOCKS + 1, dtype=np.int64) * RREAL + AGG_BLOCKS - 1) // AGG_BLOCKS
    owner = edge_rows // RREAL
    local_r = (edge_rows - owner * RREAL).astype(np.int64)
    block = (local_r * AGG_BLOCKS) // RREAL
    rel = local_r - st[block]
    c_owner = edge_cols // RREAL
    c_table = c_owner * ROWS_PER_CORE + (edge_cols - c_owner * RREAL)
    quad = c_table // QUAD_ROWS
    lcol = (c_table - quad * QUAD_ROWS).astype(np.int16)

    per_core = []
    counts = np.zeros((N_CORES, AGG_BLOCKS, N_QUAD), dtype=np.int64)
    for c in range(N_CORES):
        m = owner == c
        cell_m = block[m] * N_QUAD + quad[m]
        order = np.argsort(cell_m, kind="stable")
        per_core.append(
            (cell_m[order], lcol[m][order], rel[m][order], edge_vals[m][order])
        )
        cnt = np.bincount(cell_m, minlength=AGG_BLOCKS * N_QUAD)
        counts[c] = cnt.reshape(AGG_BLOCKS, N_QUAD)

    # uniform-across-cores tile counts per (block, quadrant)
    T = (counts.max(axis=0) + P - 1) // P  # [AGG_BLOCKS, 4]
    for b in range(AGG_BLOCKS):
        if T[b].sum() == 0:
            T[b, 0] = 1
    T = T.astype(np.int64)

    # slot ordering: supergroup-major, quadrant, block, tile
    n_tiles = int(T.sum())
    slot_of = np.zeros((AGG_BLOCKS, N_QUAD), dtype=np.int64)
    call_tiles = np.zeros((N_SG, N_QUAD), dtype=np.int64)  # tiles per (sg, q)
    s = 0
    for sg in range(N_SG):
        blo, bhi = sg * SG_BLOCKS, min((sg + 1) * SG_BLOCKS, AGG_BLOCKS)
        for q in range(N_QUAD):
            for b in range(blo, bhi):
                slot_of[b, q] = s
                s += int(T[b, q])
            call_tiles[sg, q] = int(T[blo:bhi, q].sum())
    assert s == n_tiles

    # (sg, q) stream offsets in the idx16 buffer (in int16 columns = idxs/16)
    call_off16 = np.zeros((N_SG, N_QUAD), dtype=np.int64)
    off = 0
    for sg in range(N_SG):
        for q in range(N_QUAD):
            call_off16[sg, q] = off
            off += call_tiles[sg, q] * P // 16
    total_idx16 = off

    idx_hosts, smat_hosts = [], []
    for c in range(N_CORES):
        cell_c, lcol_c, rel_c, val_c = per_core[c]
        starts = np.zeros(AGG_BLOCKS * N_QUAD + 1, dtype=np.int64)
        np.cumsum(counts[c].reshape(-1), out=starts[1:])
        n_e = len(cell_c)
        pos_in_cell = np.arange(n_e) - starts[cell_c]
        slot0 = slot_of.reshape(-1)[cell_c]  # first slot of the edge's cell
        slot_e = slot0 + (pos_in_cell >> 7)
        part_e = (pos_in_cell & 127).astype(np.int64)

        idx_slots = np.zeros((n_tiles, P), dtype=np.int16)
        idx_slots[slot_e, part_e] = lcol_c
        # scatter matrices: smat[p, slot*128 + r] = val of edge (slot, p) if rel==r
        smat = np.zeros((P, n_tiles * P), dtype=np.float16)
        smat[part_e, slot_e * P + rel_c] = val_c.astype(np.float16)

        # idx16 layout: per (sg, q) stream, wrapped into 16 partitions
        idx_host = np.zeros((16, total_idx16), dtype=np.int16)
        for sg in range(N_SG):
            blo = sg * SG_BLOCKS
            for q in range(N_QUAD):
                nt = int(call_tiles[sg, q])
                if nt == 0:
                    continue
                s0 = int(slot_of[blo, q])
                stream = idx_slots[s0 : s0 + nt].reshape(-1)
                wrapped = stream.reshape(-1, 16).T  # [16, nt*8]
                o = int(call_off16[sg, q])
                idx_host[:, o : o + nt * 8] = wrapped
        idx_hosts.append(idx_host)
        smat_hosts.append(smat)

    meta = dict(
        st=st,
        T=T,
        slot_of=slot_of,
        call_tiles=call_tiles,
        call_off16=call_off16,
        n_tiles=n_tiles,
        total_idx16=total_idx16,
    )
    in_maps = []
    for c in range(N_CORES):
        in_maps.append(
            {
                "xT": np.ascontiguousarray(
                    xT[:, :, c * ROWS_PER_CORE : (c + 1) * ROWS_PER_CORE]
                ),
                "w": w_f,
                "biasb": bias_b,
                "idx16": idx_hosts[c],
                "smat": smat_hosts[c],
            }
        )
    return meta, in_maps


def _build_program(meta):
    st = meta["st"]
    T = meta["T"]
    slot_of = meta["slot_of"]
    call_tiles = meta["call_tiles"]
    call_off16 = meta["call_off16"]
    n_tiles = meta["n_tiles"]
    total_idx16 = meta["total_idx16"]

    nc = bacc.Bacc(
        "TRN2",
        debug=False,
        num_devices=N_CORES,
        num_swdge_queues=N_QUEUES,
        dynamic_dma_scratch_size=SCRATCH,
    )

    xT_ap = nc.dram_tensor("xT", [2, P, ROWS_PER_CORE], _F16, kind="ExternalInput").ap()
    w_ap = nc.dram_tensor("w", [2, P, OUT_F], _F16, kind="ExternalInput").ap()
    biasb_ap = nc.dram_tensor("biasb", [P, OUT_F], _F16, kind="ExternalInput").ap()
    idx_ap = nc.dram_tensor("idx16", [16, total_idx16], _I16, kind="ExternalInput").ap()
    smat_ap = nc.dram_tensor("smat", [P, n_tiles * P], _F16, kind="ExternalInput").ap()
    out_ap = nc.dram_tensor("out", [RREAL, OUT_F], _F16, kind="ExternalOutput").ap()

    h_shard = nc.dram_tensor("h_shard", [ROWS_PER_CORE, OUT_F], _F16).ap()
    h_full = nc.dram_tensor("h_full", [N_PAD, OUT_F], _F16, addr_space="Shared").ap()

    XCH = ROWS_PER_CORE // XCHUNKS  # 3136 columns per xT load chunk

    with tile.TileContext(nc) as tc:
        # ---------------- phase 1: h_shard = x_shard @ W ----------------
        with (
            tc.tile_pool(name="p1", bufs=1) as p1,
            tc.tile_pool(name="p1h", bufs=4) as p1h,
            tc.tile_pool(name="p1ps", bufs=2, space="PSUM") as p1ps,
        ):
            xt_sb = [
                p1.tile([P, ROWS_PER_CORE], _F16, tag=f"xt{k}", name=f"xt_sb{k}")
                for k in range(2)
            ]
            w_sb = [
                p1.tile([P, OUT_F], _F16, tag=f"w{k}", name=f"w_sb{k}")
                for k in range(2)
            ]
            for k in range(2):
                nc.sync.dma_start(out=w_sb[k][:], in_=w_ap[k])
            for xc in range(XCHUNKS):
                for k in range(2):
                    nc.sync.dma_start(
                        out=xt_sb[k][:, xc * XCH : (xc + 1) * XCH],
                        in_=xT_ap[k][:, xc * XCH : (xc + 1) * XCH],
                    )
                for nt in range(xc * XCH // P, (xc + 1) * XCH // P):
                    ph = p1ps.tile([P, OUT_F], _F32, tag="p1ps")
                    for k in range(2):
                        nc.tensor.matmul(
                            ph[:],
                            lhsT=xt_sb[k][:, nt * P : (nt + 1) * P],
                            rhs=w_sb[k][:],
                            start=(k == 0),
                            stop=(k == 1),
                        )
                    hb = p1h.tile([P, OUT_F], _F16)
                    nc.scalar.copy(out=hb[:], in_=ph[:])
                    nc.sync.dma_start(
                        out=h_shard[nt * P : (nt + 1) * P, :], in_=hb[:]
                    )

        # ---------------- phase 2: AllGather ----------------
        nc.gpsimd.collective_compute(
            "AllGather",
            mybir.AluOpType.bypass,
            replica_groups=[list(range(N_CORES))],
            ins=[h_shard[:]],
            outs=[h_full[:]],
        )

        # ---------------- phase 3: gather + scatter-matmul ----------------
        with (
            tc.tile_pool(name="meta", bufs=1) as pm,
            tc.tile_pool(name="gat", bufs=24) as pg,
            tc.tile_pool(name="spool", bufs=2) as psp,
            tc.tile_pool(name="outsb", bufs=4) as po,
            tc.tile_pool(name="psum3", bufs=6, space="PSUM") as pp,
        ):
            idx_sb = pm.tile([P, total_idx16], _I16)
            biasb_sb = pm.tile([P, OUT_F], _F16)
            for r in range(8):
                nc.sync.dma_start(
                    out=idx_sb[16 * r : 16 * (r + 1), :], in_=idx_ap[:]
                )
            nc.sync.dma_start(out=biasb_sb[:], in_=biasb_ap[:])

            call_no = 0
            for sg in range(N_SG):
                blo = sg * SG_BLOCKS
                bhi = min(blo + SG_BLOCKS, AGG_BLOCKS)
                sg_slot0 = int(slot_of[blo, 0])
                sg_ntiles = int(T[blo:bhi].sum())
                # bulk-load this supergroup's scatter matrices
                s_sg = psp.tile([P, sg_ntiles * P], _F16, tag="smat", name=f"s_sg{sg}")
                nc.sync.dma_start(
                    out=s_sg[:],
                    in_=smat_ap[:, sg_slot0 * P : (sg_slot0 + sg_ntiles) * P],
                )
                g_calls = {}
                for q in range(N_QUAD):
                    nt_stream = int(call_tiles[sg, q])
                    o16 = int(call_off16[sg, q])
                    nchunks = (nt_stream + CALL_TILES - 1) // CALL_TILES
                    for k in range(nchunks):
                        ct = min(CALL_TILES, nt_stream - k * CALL_TILES)
                        g = pg.tile(
                            [P, ct, P], _F16, tag="gather", name=f"g_{sg}_{q}_{k}"
                        )
                        nc.gpsimd.dma_gather(
                            out_ap=g[:],
                            in_ap=h_full[q * QUAD_ROWS : (q + 1) * QUAD_ROWS, :],
                            idxs_ap=idx_sb[
                                :,
                                o16 + k * CALL_TILES * 8 : o16 + (k * CALL_TILES + ct) * 8,
                            ],
                            num_idxs=ct * P,
                            num_idxs_reg=ct * P,
                            elem_size=OUT_F,
                            queue_num=call_no % N_QUEUES,
                        )
                        call_no += 1
                        g_calls[(q, k)] = g
                for b in range(blo, bhi):
                    work = []  # (slot, gather_tile, pos_in_call)
                    for q in range(N_QUAD):
                        base = int(slot_of[b, q]) - int(slot_of[blo, q])
                        for t in range(int(T[b, q])):
                            sp = base + t
                            work.append(
                                (
                                    int(slot_of[b, q]) + t,
                                    g_calls[(q, sp // CALL_TILES)],
                                    sp % CALL_TILES,
                                )
                            )
                    ph3 = pp.tile([P, OUT_F], _F32, tag="ps", name="ph3")
                    for i, (slot, g, pos) in enumerate(work):
                        so = (slot - sg_slot0) * P
                        nc.tensor.matmul(
                            ph3[:],
                            lhsT=s_sg[:, so : so + P],
                            rhs=g[:, pos, :],
                            start=(i == 0),
                            stop=(i == len(work) - 1),
                        )
                    osb = po.tile([P, OUT_F], _F16, name="osb")
                    nc.vector.tensor_add(out=osb[:], in0=ph3[:], in1=biasb_sb[:])
                    cnt = int(st[b + 1] - st[b])
                    nc.sync.dma_start(
                        out=out_ap[int(st[b]) : int(st[b + 1]), :], in_=osb[:cnt, :]
                    )

    nc.compile()
    return nc


def kernel(x, edge_rows, edge_cols, edge_vals, weight, bias):
    x = np.asarray(x)
    edge_rows = np.asarray(edge_rows)
    edge_cols = np.asarray(edge_cols)
    edge_vals = np.asarray(edge_vals)
    weight = np.asarray(weight)
    bias = np.asarray(bias)

    meta, in_maps = _prep_host(x, edge_rows, edge_cols, edge_vals, weight, bias)

    key = meta["T"].tobytes()
    if key not in _cache:
        _cache[key] = _build_program(meta)
    nc = _cache[key]

    res = run_bass_kernel_spmd(nc, in_maps, list(range(N_CORES)), trace=TRACE)
    global LAST_EXEC_NS
    LAST_EXEC_NS = res.exec_time_ns
    out = np.concatenate([res.results[c]["out"] for c in range(N_CORES)], axis=0)
    return out.astype(np.float32)
